# revision 54
# baseline (speedup 1.0000x reference)
"""Trainium2 Bass kernel for the EnhancedGNNDetector (3x GCN + GAT + pool + MLP).

v2 strategy (8 NeuronCores, SPMD single program):
  - Nodes sharded contiguously: core c owns dsts [c*6250, (c+1)*6250).
  - Edges (with self-loops) partitioned by dst owner, sorted by dst, packed
    into 128-edge chunks quantized per (dst-block, src-half); chunk counts
    padded to the cross-core max so one static program serves all cores.
  - The one-hot aggregation matrices S (and their transposes S_T for the GAT
    attention-logit matmuls) are built on the HOST, fed as fp8 tensors:
    S resident in SBUF for all 4 layers (~118KB/partition); S_T streamed per
    span during the GAT layer.
  - Per layer: dinv-scaled node features written to a local DRAM table (fp8
    for GCN layers, fp16 for GAT), AllGathered to a full table; dma_gather
    pulls edge source rows; matmuls with stationary S accumulate per-block
    sums in PSUM.  GCN aggregation runs fp8 x fp8 with DoubleRow perf mode
    (same-block chunk pairs); the GAT runs mixed fp8-S x fp16-messages.
  - Gathers use elem_size < row stride (raw InstDMAGatherAnt emission: only
    the stride must be a 256B multiple): L1/L3 pull 128B from 256B rows,
    the GAT pulls 528B (264 slots) from 768B rows.
  - GAT: table rows are [hg fp16 (256, (f,h)-interleaved) | als fp16 (4) |
    pad].  al_d per edge comes from per-chunk matmuls with stationary S_T
    and moving ald[block]; al_s is injected with one span-wide identity
    matmul; al_s/al_d themselves come from folding a_src/a_dst into the
    weight matmul (host-precomputed Wg @ a).  leaky_relu runs on DVE
    (Exp and Lrelu share no ACT table).  exp weights are written into spare
    message columns for the denominator; messages are scaled in-place on DVE
    (the (f,h) interleave keeps the innermost stride 1 for 2x mode).
  - src index is int16 for dma_gather, so tables are gathered in two halves.
"""

import numpy as np
import ml_dtypes
import concourse.bacc as bacc
import concourse.bass as bass
import concourse.mybir as mybir
import concourse.tile as tile
from concourse.bass_utils import run_bass_kernel_spmd

F16 = np.float16
F8 = ml_dtypes.float8_e4m3
N = 50000
E = 800000
NCORES = 8
NPC = N // NCORES            # 6250 nodes per core
NB = (NPC + 127) // 128      # 49 dst blocks per core
LASTB = NPC - 128 * (NB - 1)  # 106 rows in last block
HALF = 32768                 # int16 gather split
D_IN = 128
HID = 256
H3D = 128                    # dim of the L3/GAT aggregation tables
HEADS = 4
FH = 64
GSLOT = 384                  # GAT table row slots (fp16): 256 hg + 4 als + pad
DM = HID + 8                 # GAT message cols: 256 hg + 4 als-slots + 4 exp
OUT = 8
NEG = 0.2
GBLK = 2                     # blocks per group (PSUM accumulators held at once)
MS = 20                      # max chunks per L2 gather sub-span
MS13 = 40                    # max chunks per L1/L3 gather sub-span (128B rows)
MS_GAT = 14                  # max chunks per GAT gather sub-span
PAIR = True                  # fp8 DoubleRow pairing for GCN aggregation

fp8 = mybir.dt.float8e4
fp16 = mybir.dt.float16
fp32 = mybir.dt.float32
i16 = mybir.dt.int16
ALU = mybir.AluOpType
ACT = mybir.ActivationFunctionType


# --------------------------------------------------------------------------
# host-side schedule + per-core streams
# --------------------------------------------------------------------------

def _preprocess(x, edge_index):
    src = np.concatenate([edge_index[0], np.arange(N, dtype=np.int64)])
    dst = np.concatenate([edge_index[1], np.arange(N, dtype=np.int64)])
    deg = np.bincount(dst, minlength=N).astype(np.float32)
    dinv = np.where(deg > 0, 1.0 / np.sqrt(deg), 0.0).astype(np.float32)

    order = np.argsort(dst, kind="stable")
    s_src, s_dst = src[order], dst[order]

    core = s_dst // NPC
    blk = (s_dst % NPC) // 128
    half = (s_src >= HALF).astype(np.int64)

    key = (core * NB + blk) * 2 + half
    korder = np.argsort(key, kind="stable")   # stable: keeps dst order inside
    k_src, k_dst, k_key = s_src[korder], s_dst[korder], key[korder]
    bounds = np.searchsorted(k_key, np.arange(NCORES * NB * 2 + 1))
    cnt = (bounds[1:] - bounds[:-1]).reshape(NCORES, NB, 2)
    CH = -(-cnt.max(axis=0) // 128)            # [NB, 2] chunk counts

    # canonical chunk layout: per group of GBLK blocks, lo chunks then hi
    layout = []               # per group: (lo_start, lo_n, hi_start, hi_n, blocks)
    chunk_block = []
    pos = 0
    for g0 in range(0, NB, GBLK):
        blocks = tuple(range(g0, min(g0 + GBLK, NB)))
        lo_start = pos
        for b in blocks:
            chunk_block += [b] * int(CH[b, 0])
            pos += int(CH[b, 0])
        hi_start = pos
        for b in blocks:
            chunk_block += [b] * int(CH[b, 1])
            pos += int(CH[b, 1])
        layout.append((lo_start, hi_start - lo_start, hi_start, pos - hi_start, blocks))
    NCH = pos
    NSLAB = NCH
    segs_by_chunk = [[(chunk_block[K], K)] for K in range(NCH)]
    total_per_block = (CH[:, 0] + CH[:, 1]).astype(np.int64)

    def wrap(stream):
        return np.ascontiguousarray(np.tile(stream.reshape(-1, 16).T.copy(), (8, 1)))

    # chunk start offset per (block, half)
    ch_start = np.zeros((NB, 2), np.int64)
    for K in range(NCH):
        pass
    pos2 = {}
    for (lo_s, lo_n, hi_s, hi_n, blocks) in layout:
        p = lo_s
        for b in blocks:
            ch_start[b, 0] = p
            p += int(CH[b, 0])
        p = hi_s
        for b in blocks:
            ch_start[b, 1] = p
            p += int(CH[b, 1])

    idxs_all, S_all, ST_all = [], [], []
    for c in range(NCORES):
        idx_stream = np.zeros(NCH * 128, np.int16)
        S = np.zeros((128, NSLAB, 128), np.float32)
        for b in range(NB):
            for h in (0, 1):
                k = (c * NB + b) * 2 + h
                e0, e1 = bounds[k], bounds[k + 1]
                n = e1 - e0
                if n == 0:
                    continue
                es, ed = k_src[e0:e1], k_dst[e0:e1]
                p = ch_start[b, h] * 128
                idx_stream[p:p + n] = (es - (HALF if h else 0)).astype(np.int16)
                ch_ids = ch_start[b, h] + np.arange(n) // 128
                rows = np.arange(n) % 128
                rel = (ed % NPC - b * 128).astype(np.int64)
                S[rows, ch_ids, rel] = 1.0
        idxs_all.append(wrap(idx_stream))
        S_all.append(np.ascontiguousarray(
            S.reshape(128, NSLAB * 128)).astype(F8))
        ST_all.append(np.ascontiguousarray(
            S.transpose(2, 1, 0).reshape(128, NSLAB * 128)).astype(F8))

    dinv_blocks = []
    for c in range(NCORES):
        dv = np.ones(NB * 128, np.float32)
        dv[:NPC] = dinv[c * NPC:(c + 1) * NPC]
        dinv_blocks.append(np.ascontiguousarray(dv.reshape(NB, 128).T))  # [128, NB]

    return {
        "layout": layout, "NCH": NCH, "NSLAB": NSLAB,
        "segs_by_chunk": segs_by_chunk, "total_per_block": total_per_block,
        "idxs": idxs_all, "S": S_all, "ST": ST_all, "dinv": dinv_blocks,
    }


# --------------------------------------------------------------------------
# device program
# --------------------------------------------------------------------------

def _build(sched, repeat=1, no_cc=False):
    NCH = sched["NCH"]
    NSLAB = sched["NSLAB"]
    layout = sched["layout"]
    segs_by_chunk = sched["segs_by_chunk"]
    total_per_block = sched["total_per_block"]

    nc = bacc.Bacc("TRN2", target_bir_lowering=False, debug=False,
                   num_devices=NCORES, num_swdge_queues=4)

    # ---------------- external tensors ----------------
    xs = nc.dram_tensor("xs", [NPC, D_IN], fp32, kind="ExternalInput")
    idxs_d = nc.dram_tensor("idxs_d", [128, NCH * 8], i16, kind="ExternalInput")
    S_d = nc.dram_tensor("S_d", [128, NSLAB * 128], fp8, kind="ExternalInput")
    ST_d = nc.dram_tensor("ST_d", [128, NSLAB * 128], fp8, kind="ExternalInput")
    dinv_d = nc.dram_tensor("dinv_d", [128, NB], fp32, kind="ExternalInput")
    w1_d = nc.dram_tensor("w1_d", [128, HID], fp16, kind="ExternalInput")
    w2_d = nc.dram_tensor("w2_d", [128, 2 * HID], fp16, kind="ExternalInput")
    w3_d = nc.dram_tensor("w3_d", [128, 2 * H3D], fp16, kind="ExternalInput")
    wg_d = nc.dram_tensor("wg_d", [128, HID], fp16, kind="ExternalInput")
    wgad_d = nc.dram_tensor("wgad_d", [128, 2 * HEADS], fp16, kind="ExternalInput")
    b1_d = nc.dram_tensor("b1_d", [1, HID], fp16, kind="ExternalInput")
    b2_d = nc.dram_tensor("b2_d", [1, HID], fp16, kind="ExternalInput")
    b3_d = nc.dram_tensor("b3_d", [128, H3D], fp32, kind="ExternalInput")
    bg_d = nc.dram_tensor("bg_d", [128, HID], fp16, kind="ExternalInput")
    wc1_d = nc.dram_tensor("wc1_d", [128, 2 * 128], fp32, kind="ExternalInput")
    wc2_d = nc.dram_tensor("wc2_d", [128, 64], fp32, kind="ExternalInput")
    wc3_d = nc.dram_tensor("wc3_d", [64, 8], fp32, kind="ExternalInput")
    bc1_d = nc.dram_tensor("bc1_d", [128, 1], fp32, kind="ExternalInput")
    bc2_d = nc.dram_tensor("bc2_d", [64, 1], fp32, kind="ExternalInput")
    bc3_d = nc.dram_tensor("bc3_d", [8, 1], fp32, kind="ExternalInput")
    rowmask_d = nc.dram_tensor("rowmask_d", [128, 1], fp32, kind="ExternalInput")
    out_d = nc.dram_tensor("out_d", [8, 1], fp32, kind="ExternalOutput")

    # internal DRAM tables (fp8 rows padded to 256B gather granularity)
    g1loc = nc.dram_tensor("g1loc", [NPC, 256], fp8)
    g1full = nc.dram_tensor("g1full", [N, 256], fp8, addr_space="Shared")
    g2loc = nc.dram_tensor("g2loc", [NPC, HID], fp8)
    g2full = nc.dram_tensor("g2full", [N, HID], fp8, addr_space="Shared")
    g3loc = nc.dram_tensor("g3loc", [NPC, 256], fp8)
    g3full = nc.dram_tensor("g3full", [N, 256], fp8, addr_space="Shared")
    gtloc = nc.dram_tensor("gtloc", [NPC, GSLOT], fp16)
    gtfull = nc.dram_tensor("gtfull", [N, GSLOT], fp16, addr_space="Shared")
    arin = nc.dram_tensor("arin", [128, 2], fp32)
    arout = nc.dram_tensor("arout", [128, 2], fp32, addr_space="Shared")

    RG = [list(range(NCORES))]

    with tile.TileContext(nc) as tc:
        import contextlib
        es = contextlib.ExitStack()
        with es:
            pers = es.enter_context(tc.tile_pool(name="pers", bufs=1))
            # ---------- persistent SBUF ----------
            Sres = pers.tile([128, NSLAB, 128], fp8)
            idxs = pers.tile([128, NCH * 8], i16)
            nc.sync.dma_start(idxs[:], idxs_d[:])
            dinv = pers.tile([128, NB], fp32)
            nc.sync.dma_start(dinv[:], dinv_d[:])

            w1 = pers.tile([128, HID], fp16)
            w2 = pers.tile([128, 2 * HID], fp16)
            w3 = pers.tile([128, 2 * H3D], fp16)
            wg = pers.tile([128, HID], fp16)
            wgad = pers.tile([128, 2 * HEADS], fp16)
            b1r = pers.tile([1, HID], fp16)
            b2r = pers.tile([1, HID], fp16)
            b3r = pers.tile([128, H3D], fp32)
            bgr = pers.tile([128, HID], fp16)
            wc1 = pers.tile([128, 2 * 128], fp32)
            wc2 = pers.tile([128, 64], fp32)
            wc3 = pers.tile([64, 8], fp32)
            bc1 = pers.tile([128, 1], fp32)
            bc2 = pers.tile([64, 1], fp32)
            bc3 = pers.tile([8, 1], fp32)
            rowmask = pers.tile([128, 1], fp32)

            def load_persistent():
                nc.sync.dma_start(Sres[:].rearrange("p a b -> p (a b)"), S_d[:])
                for t, d in ((w1, w1_d), (w2, w2_d), (w3, w3_d), (wg, wg_d),
                             (wgad, wgad_d), (b1r, b1_d), (b2r, b2_d),
                             (b3r, b3_d), (bgr, bg_d), (wc1, wc1_d),
                             (wc2, wc2_d), (wc3, wc3_d), (bc1, bc1_d),
                             (bc2, bc2_d), (bc3, bc3_d), (rowmask, rowmask_d)):
                    nc.sync.dma_start(t[:], d[:])

            iota_i = pers.tile([128, 128], i16)
            nc.gpsimd.iota(iota_i[:], pattern=[[1, 128]], base=0, channel_multiplier=0)
            iota_f = pers.tile([128, 128], fp16)
            nc.vector.tensor_copy(iota_f[:], iota_i[:])
            iop_i = pers.tile([128, 1], i16)
            nc.gpsimd.iota(iop_i[:], pattern=[[1, 1]], base=0, channel_multiplier=1)
            iop_f = pers.tile([128, 1], fp16)
            nc.vector.tensor_copy(iop_f[:], iop_i[:])
            ident = pers.tile([128, 128], fp16)
            nc.vector.tensor_tensor(
                ident[:], iop_f[:].broadcast_to([128, 128]), iota_f[:],
                op=ALU.is_equal)
            ones_r = pers.tile([1, 128], fp16)
            nc.vector.memset(ones_r[:], 1.0)
            ones_c = pers.tile([128, 1], fp16)
            nc.vector.memset(ones_c[:], 1.0)

            alad_all = pers.tile([128, NB, 2 * HEADS], fp32)
            als_all = alad_all[:, :, 0:HEADS]
            ald_all = alad_all[:, :, HEADS:2 * HEADS]
            ald16 = pers.tile([128, NB, HEADS], fp16)
            crep = pers.tile([128, HEADS], fp32)

            def rows(b):
                return LASTB if b == NB - 1 else 128

            # ---------- helpers ----------
            def transpose_to_sbuf(pool, psum_pool, src16, nslab, tag):
                """src16 [128, nslab*128] fp16 -> returns [128, nslab, 128] fp16."""
                out = pool.tile([128, nslab, 128], fp16, tag=tag, name=f"tT_{tag}")
                for s in range(nslab):
                    pt = psum_pool.tile([128, 128], fp16, tag="tr", name="pt_tr", bufs=2)
                    nc.tensor.transpose(pt[:], src16[:, s * 128:(s + 1) * 128], ident[:])
                    nc.scalar.copy(out[:, s, :], pt[:])
                return out

            qctr = [0]

            def next_q():
                qctr[0] += 1
                return qctr[0] % 4

            def gather_into(m_tile, table, start_chunk, n_chunks, elem, step):
                # dma_gather clone allowing elem_size < row stride (the 256B
                # multiple constraint only applies to the stride / transpose)
                eng = nc.gpsimd
                in_ap = table
                stride_bytes = step * mybir.dt.size(in_ap.dtype)
                assert stride_bytes % 256 == 0
                _in_ap = eng.lower_ap_dma(in_ap, for_custom_bir_dma=True)
                _idxs_ap = eng.lower_ap(
                    idxs[:, start_chunk * 8:(start_chunk + n_chunks) * 8])
                _out_ap = eng.lower_ap(m_tile[:, 0:n_chunks, :])
                eng.add_instruction(
                    mybir.InstDMAGatherAnt(
                        name=nc.get_next_instruction_name(),
                        ins=[*_in_ap, _idxs_ap,
                             eng.lower_val_access(eng.to_reg(n_chunks * 128))],
                        outs=[_out_ap],
                        transpose=False,
                        num_idxs=n_chunks * 128,
                        elem_size=elem,
                        stride_bytes_256=stride_bytes // 256,
                        gen_mode=0,
                        single_packet=False,
                        queue_num=next_q(),
                    ))

            def maybe_cc(kind, op, replica_groups, ins, outs):
                if no_cc:
                    nc.sync.dma_start(outs[0].tensor[0:ins[0].shape[0]], ins[0])
                else:
                    nc.gpsimd.collective_compute(kind, op, replica_groups=replica_groups,
                                                 ins=ins, outs=outs)

            def agg_matmuls(pagg_of, m_tile, start, n_ch, D, first, done,
                            pair=False):
                """Emit segment matmuls for chunks [start, start+n_ch).
                pair=True fuses same-block chunk pairs with fp8 DoubleRow."""
                kk = 0
                while kk < n_ch:
                    K = start + kk
                    b, si = segs_by_chunk[K][0]
                    if (pair and kk + 1 < n_ch
                            and segs_by_chunk[K + 1][0][0] == b):
                        done[b] += 2
                        nc.tensor.matmul(
                            pagg_of[b][:],
                            Sres[:, si:si + 2, :], m_tile[:, kk:kk + 2, 0:D],
                            start=first[b], stop=(done[b] == total_per_block[b]),
                            perf_mode=mybir.MatmulPerfMode.DoubleRow)
                        first[b] = False
                        kk += 2
                        continue
                    done[b] += 1
                    nc.tensor.matmul(
                        pagg_of[b][:], Sres[:, si, :], m_tile[:, kk, 0:D],
                        start=first[b], stop=(done[b] == total_per_block[b]))
                    first[b] = False
                    kk += 1

            def run_body(rep):
                # ================= phase 0: g1 = dinv * x =================
                NFB = NB - 1          # full 128-row blocks
                with tc.tile_pool(name=f"p0_{rep}", bufs=1) as p0:
                    xt = p0.tile([128, NB, D_IN], fp32, name="xt")
                    nc.sync.dma_start(
                        xt[:, 0:NFB, :],
                        xs[0:NFB * 128, :].rearrange("(b p) d -> p b d", p=128))
                    nc.sync.dma_start(xt[:LASTB, NFB, :], xs[NFB * 128:NPC, :])
                    gt = p0.tile([128, NB, D_IN], fp8, name="gt")
                    nc.vector.tensor_tensor(
                        gt[:], xt[:],
                        dinv[:].unsqueeze(2).broadcast_to([128, NB, D_IN]),
                        op=ALU.mult)
                    nc.scalar.dma_start(
                        g1loc[0:NFB * 128, 0:D_IN].rearrange("(b p) d -> p b d", p=128),
                        gt[:, 0:NFB, :])
                    nc.scalar.dma_start(
                        g1loc[NFB * 128:NPC, 0:D_IN], gt[:LASTB, NFB, :])

                maybe_cc("AllGather", ALU.bypass, RG, [g1loc[:]], [g1full[:]])
                if rep == 0:
                    load_persistent()

                # ================= GCN layer runner =================
                h1_pool = tc.tile_pool(name=f"h1pool_{rep}", bufs=1)
                h1_ctx = h1_pool.__enter__()
                h1_all = h1_ctx.tile([128, NB, HID], fp16)

                def gcn_layer(lname, table_full, Dtab, Dstep, D, evict_fn, mbufs=5, ms=MS):
                    with (tc.tile_pool(name=f"{lname}_sb_{rep}", bufs=2) as lp,
                          tc.tile_pool(name=f"{lname}_ps_{rep}", bufs=4, space="PSUM") as pp,
                          tc.tile_pool(name=f"{lname}_wps_{rep}", bufs=2, space="PSUM") as wp):
                        tab_lo = table_full[0:HALF, :]
                        tab_hi = table_full[HALF:N, :]
                        for (lo_s, lo_n, hi_s, hi_n, blocks) in layout:
                            paggs = {}
                            for b in blocks:
                                paggs[b] = pp.tile([128, D], fp32, tag="agg", name="pagg")
                            first = {b: True for b in blocks}
                            done = {b: 0 for b in blocks}
                            for (start, n_ch, tab) in ((lo_s, lo_n, tab_lo), (hi_s, hi_n, tab_hi)):
                                for s0 in range(0, n_ch, ms):
                                    ns = min(ms, n_ch - s0)
                                    m = lp.tile([128, ns, Dtab], fp8, tag="m", name="m", bufs=mbufs)
                                    gather_into(m, tab, start + s0, ns, Dtab, Dstep)
                                    agg_matmuls(paggs, m, start + s0, ns, D, first, done, pair=PAIR)
                            for b in blocks:
                                evict_fn(b, paggs[b], lp, wp)

                # ---------- layer 1 ----------
                def evict1(b, pagg, lp, wp):
                    r = rows(b)
                    a1s = lp.tile([128, D_IN], fp16, tag="ev1", name="a1s")
                    nc.vector.tensor_scalar(a1s[:], pagg[:], dinv[:, b:b + 1], None, op0=ALU.mult)
                    a1T = transpose_to_sbuf(lp, wp, a1s, 1, "ev1T")
                    ph = wp.tile([128, HID], fp32, tag="wout", name="ph1", bufs=2)
                    nc.tensor.matmul(ph[:], a1T[:, 0, :], w1[:], start=True, stop=False)
                    nc.tensor.matmul(ph[:], ones_r[:], b1r[:], start=False, stop=True)
                    h1t = h1_all[:, b, :]
                    nc.scalar.activation(h1t, ph[:], ACT.Relu)
                    g2t = lp.tile([128, HID], fp8, tag="ev1g", name="g2t")
                    nc.vector.tensor_scalar(g2t[:], h1t, dinv[:, b:b + 1], None, op0=ALU.mult)
                    nc.scalar.dma_start(g2loc[b * 128:b * 128 + r, :], g2t[:r, :])

                gcn_layer("L1", g1full, D_IN, 256, D_IN, evict1, ms=MS13)
                maybe_cc("AllGather", ALU.bypass, RG, [g2loc[:]], [g2full[:]])

                # ---------- layer 2 (+ residual + L3 transform) ----------
                def evict2(b, pagg, lp, wp):
                    r = rows(b)
                    a2s = lp.tile([128, HID], fp16, tag="ev2", name="a2s")
                    nc.vector.tensor_scalar(a2s[:], pagg[:], dinv[:, b:b + 1], None, op0=ALU.mult)
                    a2T = transpose_to_sbuf(lp, wp, a2s, 2, "ev2T")
                    ph = wp.tile([128, HID], fp32, tag="wout", name="ph2", bufs=2)
                    nc.tensor.matmul(ph[:], a2T[:, 0, :], w2[:, 0:HID], start=True, stop=False)
                    nc.tensor.matmul(ph[:], a2T[:, 1, :], w2[:, HID:2 * HID], start=False, stop=False)
                    nc.tensor.matmul(ph[:], ones_r[:], b2r[:], start=False, stop=True)
                    r2 = lp.tile([128, HID], fp16, tag="ev2r", name="r2")
                    nc.scalar.activation(r2[:], ph[:], ACT.Relu)
                    h2t16 = lp.tile([128, HID], fp16, tag="ev2h6", name="h2t16")
                    nc.vector.tensor_tensor(h2t16[:], r2[:], h1_all[:, b, :], op=ALU.add)
                    h2T = transpose_to_sbuf(lp, wp, h2t16, 2, "ev2hT")
                    pt3 = wp.tile([128, H3D], fp32, tag="wout", name="pt3", bufs=2)
                    nc.tensor.matmul(pt3[:], h2T[:, 0, :], w3[:, 0:H3D], start=True, stop=False)
                    nc.tensor.matmul(pt3[:], h2T[:, 1, :], w3[:, H3D:2 * H3D], start=False, stop=True)
                    g3t = lp.tile([128, H3D], fp8, tag="ev2g", name="g3t")
                    nc.vector.tensor_scalar(g3t[:], pt3[:], dinv[:, b:b + 1], None, op0=ALU.mult)
                    nc.scalar.dma_start(g3loc[b * 128:b * 128 + r, 0:H3D], g3t[:r, :])

                gcn_layer("L2", g2full, HID, HID, HID, evict2, mbufs=4)
                h1_pool.__exit__(None, None, None)
                maybe_cc("AllGather", ALU.bypass, RG, [g3loc[:]], [g3full[:]])

                # ---------- layer 3 aggregation + GAT prep ----------
                def evict3(b, pagg, lp, wp):
                    r = rows(b)
                    a3s = lp.tile([128, H3D], fp32, tag="ev3", name="a3s")
                    nc.vector.tensor_scalar(a3s[:], pagg[:], dinv[:, b:b + 1], None, op0=ALU.mult)
                    a3b = lp.tile([128, H3D], fp32, tag="ev3b", name="a3b")
                    nc.vector.tensor_tensor(a3b[:], a3s[:], b3r[:], op=ALU.add)
                    h3t16 = lp.tile([128, H3D], fp16, tag="ev3h", name="h3t16")
                    nc.scalar.activation(h3t16[:], a3b[:], ACT.Relu)
                    h3T = transpose_to_sbuf(lp, wp, h3t16, 1, "ev3T")
                    # hg plus al_s / al_d (Wg @ a_src/a_dst) in one psum tile
                    phg = wp.tile([128, HID + 2 * HEADS], fp32, tag="wout", name="phg", bufs=2)
                    nc.tensor.matmul(phg[:, 0:HID], h3T[:, 0, :], wg[:], start=True, stop=True)
                    nc.tensor.matmul(phg[:, HID:], h3T[:, 0, :], wgad[:], start=True, stop=True)
                    nc.vector.tensor_copy(alad_all[:, b, :], phg[:, HID:])
                    # table tile: [hg fp16 (f,h) | als fp16 | pad]
                    tabt = lp.tile([128, GSLOT], fp16, tag="ev3tab", name="tabt")
                    nc.scalar.copy(tabt[:, 0:HID], phg[:, 0:HID])
                    nc.scalar.copy(tabt[:, HID:HID + HEADS], phg[:, HID:HID + HEADS])
                    nc.scalar.dma_start(gtloc[b * 128:b * 128 + r, :], tabt[:r, :])

                gcn_layer("L3", g3full, H3D, 256, H3D, evict3, ms=MS13)
                nc.vector.tensor_copy(ald16[:], ald_all)

                # shift constants c[h] = leaky(max al_s + max al_d)
                cps = contextlib.ExitStack()
                cp = cps.enter_context(tc.tile_pool(name=f"cp_{rep}", bufs=1))
                cpp = cps.enter_context(tc.tile_pool(name=f"cpp_{rep}", bufs=1, space="PSUM"))
                m1 = cp.tile([128, HEADS], fp32)
                nc.vector.tensor_reduce(
                    m1[:], als_all.rearrange("p b h -> p h b"),
                    axis=mybir.AxisListType.X, op=ALU.max)
                m2 = cp.tile([128, HEADS], fp32)
                nc.vector.tensor_reduce(
                    m2[:], ald_all.rearrange("p b h -> p h b"),
                    axis=mybir.AxisListType.X, op=ALU.max)
                m1_16 = cp.tile([128, HEADS], fp16)
                nc.vector.tensor_copy(m1_16[:], m1[:])
                m2_16 = cp.tile([128, HEADS], fp16)
                nc.vector.tensor_copy(m2_16[:], m2[:])
                pmt1 = cpp.tile([HEADS, 128], fp16, tag="pmt1", name="pmt1")
                nc.tensor.transpose(pmt1[:], m1_16[:], ident[:])
                pmt2 = cpp.tile([HEADS, 128], fp16, tag="pmt2", name="pmt2")
                nc.tensor.transpose(pmt2[:], m2_16[:], ident[:])
                mt = cp.tile([HEADS, 2 * 128], fp32)
                nc.scalar.copy(mt[:, 0:128], pmt1[:])
                nc.scalar.copy(mt[:, 128:256], pmt2[:])
                ms = cp.tile([HEADS, 2], fp32)
                nc.vector.tensor_reduce(
                    ms[:], mt[:].rearrange("p (a j) -> p a j", a=2),
                    axis=mybir.AxisListType.X, op=ALU.max)
                ub = cp.tile([HEADS, 1], fp32)
                nc.vector.tensor_tensor(ub[:], ms[:, 0:1], ms[:, 1:2], op=ALU.add)
                ub2 = cp.tile([HEADS, 1], fp32)
                nc.vector.tensor_scalar(ub2[:], ub[:], 0.2, None, op0=ALU.mult)
                cc = cp.tile([HEADS, 1], fp32)
                nc.vector.tensor_tensor(cc[:], ub[:], ub2[:], op=ALU.max)
                cc16 = cp.tile([HEADS, 1], fp16)
                nc.vector.tensor_copy(cc16[:], cc[:])
                pcr = cpp.tile([1, HEADS], fp16)
                nc.tensor.transpose(pcr[:], cc16[:HEADS, :], ident[0:HEADS, 0:HEADS])
                pcr_sb = cp.tile([1, HEADS], fp16)
                nc.scalar.copy(pcr_sb[:], pcr[:])
                pcrep = cpp.tile([128, HEADS], fp32)
                nc.tensor.matmul(pcrep[:], ones_r[:], pcr_sb[:], start=True, stop=True)
                nc.scalar.copy(crep[:], pcrep[:])
                cps.close()

                maybe_cc("AllGather", ALU.bypass, RG, [gtloc[:]], [gtfull[:]])

                # ================= GAT layer =================
                plp_cm = tc.tile_pool(name=f"pool_ps_{rep}", bufs=1, space="PSUM")
                plp = plp_cm.__enter__()
                ppool = plp.tile([128, 2], fp32, tag="pp", name="ppool")
                ppool0 = ppool[:, 0:1]
                ppool1 = ppool[:, 1:2]
                with (tc.tile_pool(name=f"gat_sb_{rep}", bufs=2) as gp,
                      tc.tile_pool(name=f"gat_ps_{rep}", bufs=4, space="PSUM") as gpp,
                      tc.tile_pool(name=f"gat_ups_{rep}", bufs=3, space="PSUM") as upp):
                    tab_lo = gtfull[0:HALF, :]
                    tab_hi = gtfull[HALF:N, :]
                    first = {b: True for b in range(NB)}
                    done = {b: 0 for b in range(NB)}
                    paggs = {}

                    def gat_span(start, n_ch, tab):
                        if n_ch == 0:
                            return
                        m = gp.tile([128, n_ch, DM], fp16, tag="gm", name="gm", bufs=6)
                        gather_into(m, tab, start, n_ch, DM, GSLOT)
                        # stream S_T slabs for this span
                        si0 = segs_by_chunk[start][0][1]
                        si1 = segs_by_chunk[start + n_ch - 1][-1][1] + 1
                        nsl = si1 - si0
                        st = gp.tile([128, nsl, 128], fp8, tag="gst", name="gst", bufs=4)
                        nc.sync.dma_start(
                            st[:].rearrange("p a b -> p (a b)"),
                            ST_d[:, si0 * 128:si1 * 128])
                        # u[e, h] = als[e, h] + ald[dst_e, h] via PE
                        ups = upp.tile([128, n_ch, HEADS], fp32, tag="ups", name="ups")
                        nc.tensor.matmul(
                            ups[:], ident[:], m[:, :, HID:HID + HEADS],
                            start=True, stop=False, skip_group_check=True)
                        for kk in range(n_ch):
                            segs = segs_by_chunk[start + kk]
                            for j, (b, si) in enumerate(segs):
                                nc.tensor.matmul(
                                    ups[:, kk, :], st[:, si - si0, :], ald16[:, b, :],
                                    start=False, stop=(j == len(segs) - 1),
                                    skip_group_check=True)
                        # leaky_relu on DVE (keeps ACT on the exp table), shift
                        lr2 = gp.tile([128, n_ch, HEADS], fp32, tag="glr2", name="glr2")
                        nc.vector.tensor_scalar(
                            lr2[:], ups[:], NEG, None, op0=ALU.mult)
                        lr = gp.tile([128, n_ch, HEADS], fp32, tag="glr", name="glr")
                        nc.vector.tensor_tensor(lr[:], ups[:], lr2[:], op=ALU.max)
                        lsh = gp.tile([128, n_ch, HEADS], fp32, tag="glsh", name="glsh")
                        nc.vector.tensor_tensor(
                            lsh[:], lr[:],
                            crep[:].unsqueeze(1).broadcast_to([128, n_ch, HEADS]),
                            op=ALU.subtract)
                        # exp weights straight into the den message cols
                        nc.scalar.activation(
                            m[:, :, HID + HEADS:HID + 2 * HEADS],
                            lsh[:].rearrange("p k h -> p (k h)"), ACT.Exp)
                        nc.vector.tensor_tensor(
                            m[:, :, 0:HID].rearrange("p k (f h) -> p k f h", h=HEADS),
                            m[:, :, 0:HID].rearrange("p k (f h) -> p k f h", h=HEADS),
                            m[:, :, HID + HEADS:HID + 2 * HEADS]
                                .unsqueeze(2).broadcast_to([128, n_ch, FH, HEADS]),
                            op=ALU.mult)
                        agg_matmuls(paggs, m, start, n_ch, DM, first, done)

                    def gat_evict(b):
                        r = rows(b)
                        pg = paggs.pop(b)
                        den = gp.tile([128, HEADS], fp32, tag="gden", name="gden")
                        nc.scalar.copy(den[:], pg[:, HID + HEADS:HID + 2 * HEADS])
                        nc.vector.tensor_scalar(den[:], den[:], 1e-30, None, op0=ALU.max)
                        rden = gp.tile([128, HEADS], fp32, tag="grden", name="grden")
                        nc.vector.reciprocal(rden[:], den[:])
                        t1 = gp.tile([128, HID], fp16, tag="gt1", name="gt1")
                        nc.vector.tensor_tensor(
                            t1[:].rearrange("p (f h) -> p f h", h=HEADS),
                            pg[:, 0:HID].rearrange("p (f h) -> p f h", h=HEADS),
                            rden[:].unsqueeze(1).broadcast_to([128, FH, HEADS]),
                            op=ALU.mult)
                        t2 = gp.tile([128, HID], fp16, tag="gt2", name="gt2")
                        nc.vector.tensor_tensor(t2[:], t1[:], bgr[:], op=ALU.add)
                        hatt = gp.tile([128, HID], fp16, tag="ghat", name="ghat")
                        nc.scalar.activation(hatt[:], t2[:], ACT.Relu)
                        if r < 128:
                            nc.vector.tensor_scalar(hatt[:], hatt[:], rowmask[:], None, op0=ALU.mult)
                        nc.tensor.matmul(ppool0, hatt[:, 0:128], ones_c[:],
                                         start=(b == 0), stop=(b == NB - 1))
                        nc.tensor.matmul(ppool1, hatt[:, 128:256], ones_c[:],
                                         start=(b == 0), stop=(b == NB - 1))

                    for (lo_s, lo_n, hi_s, hi_n, blocks) in layout:
                        for b in blocks:
                            paggs[b] = gpp.tile([128, DM], fp32, tag="gagg", name="gagg")
                        for (start, n_ch, tab) in ((lo_s, lo_n, tab_lo), (hi_s, hi_n, tab_hi)):
                            for s0 in range(0, n_ch, MS_GAT):
                                gat_span(start + s0, min(MS_GAT, n_ch - s0), tab)
                        for b in blocks:
                            gat_evict(b)

                # ---------- pooling + AllReduce + MLP ----------
                with (tc.tile_pool(name=f"mlp_sb_{rep}", bufs=1) as mp,
                      tc.tile_pool(name=f"mlp_ps_{rep}", bufs=1, space="PSUM") as mpp):
                    pool_sb = mp.tile([128, 2], fp32, name="pool_sb")
                    nc.scalar.copy(pool_sb[:], ppool[:])
                    nc.sync.dma_start(arin[:], pool_sb[:])
                    maybe_cc("AllReduce", ALU.add, RG, [arin[:]], [arout[:]])
                    pooled = mp.tile([128, 2], fp32, name="pooled")
                    nc.sync.dma_start(pooled[:], arout[:])
                    nc.vector.tensor_scalar(pooled[:], pooled[:], 1.0 / N, None, op0=ALU.mult)
                    pz1 = mpp.tile([128, 1], fp32, tag="pz", name="pz1")
                    nc.tensor.matmul(pz1[:], wc1[:, 0:128], pooled[:, 0:1], start=True, stop=False)
                    nc.tensor.matmul(pz1[:], wc1[:, 128:256], pooled[:, 1:2], start=False, stop=True)
                    z1 = mp.tile([128, 1], fp32, name="z1")
                    nc.scalar.activation(z1[:], pz1[:], ACT.Relu, bias=bc1[:])
                    pz2 = mpp.tile([64, 1], fp32, tag="pz", name="pz2")
                    nc.tensor.matmul(pz2[:], wc2[:], z1[:], start=True, stop=True)
                    z2 = mp.tile([64, 1], fp32, name="z2")
                    nc.scalar.activation(z2[:], pz2[:], ACT.Relu, bias=bc2[:])
                    pz3 = mpp.tile([8, 1], fp32, tag="pz", name="pz3")
                    nc.tensor.matmul(pz3[:], wc3[:], z2[:64, :], start=True, stop=True)
                    zo = mp.tile([8, 1], fp32, name="zo")
                    nc.scalar.activation(zo[:], pz3[:], ACT.Identity, bias=bc3[:])
                    nc.sync.dma_start(out_d[:], zo[:])
                plp_cm.__exit__(None, None, None)

            for _rep in range(repeat):
                run_body(_rep)

    nc.compile()
    return nc


# --------------------------------------------------------------------------
# entry point
# --------------------------------------------------------------------------

def _fh_interleave(w):
    """Reorder columns from (h, f) to (f, h): col f*H+h <- col h*FH+f."""
    M = w.reshape(w.shape[0], HEADS, FH)
    return np.ascontiguousarray(M.transpose(0, 2, 1).reshape(w.shape[0], HEADS * FH))


def kernel(**inputs):
    x = np.asarray(inputs["x"], dtype=np.float32)
    ei = np.asarray(inputs["edge_index"], dtype=np.int64)
    sched = _preprocess(x, ei)
    nc = _build(sched)

    W = {k: np.asarray(v, dtype=np.float32) for k, v in inputs.items()
         if k not in ("x", "edge_index")}

    def pack_k(w, nslab):   # [K, M] -> [128, nslab*M] (row-slab packed)
        K, M = w.shape
        out = np.zeros((128, nslab * M), np.float32)
        for s in range(nslab):
            r0 = s * 128
            r1 = min(K, r0 + 128)
            out[0:r1 - r0, s * M:(s + 1) * M] = w[r0:r1]
        return out

    # (f, h)-interleaved GAT weights
    wg_i = _fh_interleave(W["Wg"])                              # [128, 256]
    bg_i = _fh_interleave(W["bg"].reshape(1, HEADS * FH))
    wc1_i = _fh_interleave(W["Wc1"].T).T                        # rows reordered

    common = {
        "w1_d": pack_k(W["W1"], 1).astype(F16),
        "w2_d": pack_k(W["W2"], 2).astype(F16),
        "w3_d": pack_k(W["W3"], 2).astype(F16),
        "wg_d": pack_k(wg_i, 1).astype(F16),
        "b1_d": W["b1"].reshape(1, -1).astype(F16),
        "b2_d": W["b2"].reshape(1, -1).astype(F16),
        "b3_d": np.tile(W["b3"].reshape(1, -1), (128, 1)).astype(np.float32),
        "bg_d": np.tile(bg_i, (128, 1)).astype(F16),
        "wgad_d": np.concatenate([
            (W["Wg"].reshape(D_IN, HEADS, FH) * W["a_src"][None]).sum(-1),
            (W["Wg"].reshape(D_IN, HEADS, FH) * W["a_dst"][None]).sum(-1),
        ], axis=1).astype(F16),
        "wc1_d": pack_k(wc1_i, 2).astype(np.float32),
        "wc2_d": pack_k(W["Wc2"], 1)[:, :64].astype(np.float32),
        "wc3_d": pack_k(W["Wc3"], 1)[:64, :8].astype(np.float32),
        "bc1_d": W["bc1"].reshape(-1, 1).astype(np.float32),
        "bc2_d": W["bc2"].reshape(-1, 1).astype(np.float32),
        "bc3_d": W["bc3"].reshape(-1, 1).astype(np.float32),
        "rowmask_d": (np.arange(128) < LASTB).astype(np.float32).reshape(128, 1),
    }

    in_maps = []
    for c in range(NCORES):
        in_maps.append(dict(
            common,
            xs=np.ascontiguousarray(x[c * NPC:(c + 1) * NPC]),
            idxs_d=sched["idxs"][c],
            S_d=sched["S"][c],
            ST_d=sched["ST"][c],
            dinv_d=sched["dinv"][c],
        ))

    res = run_bass_kernel_spmd(nc, in_maps, core_ids=list(range(NCORES)))
    global LAST_RESULT
    LAST_RESULT = res
    return res.results[0]["out_d"].reshape(1, OUT).astype(np.float32)


LAST_RESULT = None


# revision 56
# speedup vs baseline: 1.0457x; 1.0457x over previous
"""Trainium2 Bass kernel for the EnhancedGNNDetector (3x GCN + GAT + pool + MLP).

v2 strategy (8 NeuronCores, SPMD single program):
  - Nodes sharded contiguously: core c owns dsts [c*6250, (c+1)*6250).
  - Edges (with self-loops) partitioned by dst owner, sorted by dst, packed
    into 128-edge chunks quantized per (dst-block, src-half); chunk counts
    padded to the cross-core max so one static program serves all cores.
  - The one-hot aggregation matrices S (and their transposes S_T for the GAT
    attention-logit matmuls) are built on the HOST, fed as fp8 tensors:
    S resident in SBUF for all 4 layers (~118KB/partition); S_T streamed per
    span during the GAT layer.
  - Per layer: dinv-scaled node features written to a local DRAM table (fp8
    for GCN layers, fp16 for GAT), AllGathered to a full table; dma_gather
    pulls edge source rows; matmuls with stationary S accumulate per-block
    sums in PSUM.  GCN aggregation runs fp8 x fp8 with DoubleRow perf mode
    (same-block chunk pairs); the GAT runs mixed fp8-S x fp16-messages.
  - Gathers use elem_size < row stride (raw InstDMAGatherAnt emission: only
    the stride must be a 256B multiple): L1/L3 pull 128B from 256B rows,
    the GAT pulls 528B (264 slots) from 768B rows.
  - GAT: table rows are [hg fp16 (256, (f,h)-interleaved) | als fp16 (4) |
    pad].  al_d per edge comes from per-chunk matmuls with stationary S_T
    and moving ald[block]; al_s is injected with one span-wide identity
    matmul; al_s/al_d themselves come from folding a_src/a_dst into the
    weight matmul (host-precomputed Wg @ a).  leaky_relu runs on DVE
    (Exp and Lrelu share no ACT table).  exp weights are written into spare
    message columns for the denominator; messages are scaled in-place on DVE
    (the (f,h) interleave keeps the innermost stride 1 for 2x mode).
  - src index is int16 for dma_gather, so tables are gathered in two halves.
"""

import numpy as np
import ml_dtypes
import concourse.bacc as bacc
import concourse.bass as bass
import concourse.mybir as mybir
import concourse.tile as tile
from concourse.bass_utils import run_bass_kernel_spmd

F16 = np.float16
F8 = ml_dtypes.float8_e4m3
N = 50000
E = 800000
NCORES = 8
NPC = N // NCORES            # 6250 nodes per core
NB = (NPC + 127) // 128      # 49 dst blocks per core
LASTB = NPC - 128 * (NB - 1)  # 106 rows in last block
HALF = 32768                 # int16 gather split
D_IN = 128
HID = 256
H3D = 128                    # dim of the L3/GAT aggregation tables
HEADS = 4
FH = 64
GSLOT = 384                  # GAT table row slots (fp16): 256 hg + 4 als + pad
DM = HID + 8                 # GAT message cols: 256 hg + 4 als-slots + 4 exp
OUT = 8
NEG = 0.2
GBLK = 2                     # blocks per group (PSUM accumulators held at once)
MS = 20                      # max chunks per L2 gather sub-span
MS13 = 40                    # max chunks per L1/L3 gather sub-span (128B rows)
MS_GAT = 14                  # max chunks per GAT gather sub-span
PAIR = True                  # fp8 DoubleRow pairing for GCN aggregation

fp8 = mybir.dt.float8e4
fp16 = mybir.dt.float16
fp32 = mybir.dt.float32
i16 = mybir.dt.int16
ALU = mybir.AluOpType
ACT = mybir.ActivationFunctionType


# --------------------------------------------------------------------------
# host-side schedule + per-core streams
# --------------------------------------------------------------------------

def _preprocess(x, edge_index):
    src = np.concatenate([edge_index[0], np.arange(N, dtype=np.int64)])
    dst = np.concatenate([edge_index[1], np.arange(N, dtype=np.int64)])
    deg = np.bincount(dst, minlength=N).astype(np.float32)
    dinv = np.where(deg > 0, 1.0 / np.sqrt(deg), 0.0).astype(np.float32)

    order = np.argsort(dst, kind="stable")
    s_src, s_dst = src[order], dst[order]

    core = s_dst // NPC
    blk = (s_dst % NPC) // 128
    half = (s_src >= HALF).astype(np.int64)

    key = (core * NB + blk) * 2 + half
    korder = np.argsort(key, kind="stable")   # stable: keeps dst order inside
    k_src, k_dst, k_key = s_src[korder], s_dst[korder], key[korder]
    bounds = np.searchsorted(k_key, np.arange(NCORES * NB * 2 + 1))
    cnt = (bounds[1:] - bounds[:-1]).reshape(NCORES, NB, 2)
    CH = -(-cnt.max(axis=0) // 128)            # [NB, 2] chunk counts

    # canonical chunk layout: per group of GBLK blocks, lo chunks then hi
    layout = []               # per group: (lo_start, lo_n, hi_start, hi_n, blocks)
    chunk_block = []
    pos = 0
    for g0 in range(0, NB, GBLK):
        blocks = tuple(range(g0, min(g0 + GBLK, NB)))
        lo_start = pos
        for b in blocks:
            chunk_block += [b] * int(CH[b, 0])
            pos += int(CH[b, 0])
        hi_start = pos
        for b in blocks:
            chunk_block += [b] * int(CH[b, 1])
            pos += int(CH[b, 1])
        layout.append((lo_start, hi_start - lo_start, hi_start, pos - hi_start, blocks))
    NCH = pos
    NSLAB = NCH
    segs_by_chunk = [[(chunk_block[K], K)] for K in range(NCH)]
    total_per_block = (CH[:, 0] + CH[:, 1]).astype(np.int64)

    def wrap(stream):
        return np.ascontiguousarray(np.tile(stream.reshape(-1, 16).T.copy(), (8, 1)))

    # chunk start offset per (block, half)
    ch_start = np.zeros((NB, 2), np.int64)
    for K in range(NCH):
        pass
    pos2 = {}
    for (lo_s, lo_n, hi_s, hi_n, blocks) in layout:
        p = lo_s
        for b in blocks:
            ch_start[b, 0] = p
            p += int(CH[b, 0])
        p = hi_s
        for b in blocks:
            ch_start[b, 1] = p
            p += int(CH[b, 1])

    idxs_all, S_all, ST_all = [], [], []
    for c in range(NCORES):
        idx_stream = np.zeros(NCH * 128, np.int16)
        S = np.zeros((128, NSLAB, 128), np.float32)
        for b in range(NB):
            for h in (0, 1):
                k = (c * NB + b) * 2 + h
                e0, e1 = bounds[k], bounds[k + 1]
                n = e1 - e0
                if n == 0:
                    continue
                es, ed = k_src[e0:e1], k_dst[e0:e1]
                p = ch_start[b, h] * 128
                idx_stream[p:p + n] = (es - (HALF if h else 0)).astype(np.int16)
                ch_ids = ch_start[b, h] + np.arange(n) // 128
                rows = np.arange(n) % 128
                rel = (ed % NPC - b * 128).astype(np.int64)
                S[rows, ch_ids, rel] = 1.0
        idxs_all.append(wrap(idx_stream))
        S_all.append(np.ascontiguousarray(
            S.reshape(128, NSLAB * 128)).astype(F8))
        ST_all.append(np.ascontiguousarray(
            S.transpose(2, 1, 0).reshape(128, NSLAB * 128)).astype(F8))

    dinv_blocks = []
    for c in range(NCORES):
        dv = np.ones(NB * 128, np.float32)
        dv[:NPC] = dinv[c * NPC:(c + 1) * NPC]
        dinv_blocks.append(np.ascontiguousarray(dv.reshape(NB, 128).T))  # [128, NB]

    return {
        "layout": layout, "NCH": NCH, "NSLAB": NSLAB,
        "segs_by_chunk": segs_by_chunk, "total_per_block": total_per_block,
        "idxs": idxs_all, "S": S_all, "ST": ST_all, "dinv": dinv_blocks,
    }


# --------------------------------------------------------------------------
# device program
# --------------------------------------------------------------------------

def _build(sched, repeat=1, no_cc=False):
    NCH = sched["NCH"]
    NSLAB = sched["NSLAB"]
    layout = sched["layout"]
    segs_by_chunk = sched["segs_by_chunk"]
    total_per_block = sched["total_per_block"]

    nc = bacc.Bacc("TRN2", target_bir_lowering=False, debug=False,
                   num_devices=NCORES, num_swdge_queues=4)

    # ---------------- external tensors ----------------
    xs = nc.dram_tensor("xs", [NPC, D_IN], fp32, kind="ExternalInput")
    idxs_d = nc.dram_tensor("idxs_d", [128, NCH * 8], i16, kind="ExternalInput")
    S_d = nc.dram_tensor("S_d", [128, NSLAB * 128], fp8, kind="ExternalInput")
    ST_d = nc.dram_tensor("ST_d", [128, NSLAB * 128], fp8, kind="ExternalInput")
    dinv_d = nc.dram_tensor("dinv_d", [128, NB], fp32, kind="ExternalInput")
    w1_d = nc.dram_tensor("w1_d", [128, HID], fp16, kind="ExternalInput")
    w2_d = nc.dram_tensor("w2_d", [128, 2 * HID], fp16, kind="ExternalInput")
    w3_d = nc.dram_tensor("w3_d", [128, 2 * H3D], fp16, kind="ExternalInput")
    wg_d = nc.dram_tensor("wg_d", [128, HID], fp16, kind="ExternalInput")
    wgad_d = nc.dram_tensor("wgad_d", [128, 2 * HEADS], fp16, kind="ExternalInput")
    b1_d = nc.dram_tensor("b1_d", [1, HID], fp16, kind="ExternalInput")
    b2_d = nc.dram_tensor("b2_d", [1, HID], fp16, kind="ExternalInput")
    b3_d = nc.dram_tensor("b3_d", [128, H3D], fp32, kind="ExternalInput")
    bg_d = nc.dram_tensor("bg_d", [128, HID], fp16, kind="ExternalInput")
    wc1_d = nc.dram_tensor("wc1_d", [128, 2 * 128], fp32, kind="ExternalInput")
    wc2_d = nc.dram_tensor("wc2_d", [128, 64], fp32, kind="ExternalInput")
    wc3_d = nc.dram_tensor("wc3_d", [64, 8], fp32, kind="ExternalInput")
    bc1_d = nc.dram_tensor("bc1_d", [128, 1], fp32, kind="ExternalInput")
    bc2_d = nc.dram_tensor("bc2_d", [64, 1], fp32, kind="ExternalInput")
    bc3_d = nc.dram_tensor("bc3_d", [8, 1], fp32, kind="ExternalInput")
    rowmask_d = nc.dram_tensor("rowmask_d", [128, 1], fp32, kind="ExternalInput")
    out_d = nc.dram_tensor("out_d", [8, 1], fp32, kind="ExternalOutput")

    # internal DRAM tables (fp8 rows padded to 256B gather granularity)
    g1loc = nc.dram_tensor("g1loc", [NPC, 256], fp8)
    g1full = nc.dram_tensor("g1full", [N, 256], fp8, addr_space="Shared")
    g2loc = nc.dram_tensor("g2loc", [NPC, HID], fp8)
    g2full = nc.dram_tensor("g2full", [N, HID], fp8, addr_space="Shared")
    g3loc = nc.dram_tensor("g3loc", [NPC, 256], fp8)
    g3full = nc.dram_tensor("g3full", [N, 256], fp8, addr_space="Shared")
    gtloc = nc.dram_tensor("gtloc", [NPC, GSLOT], fp16)
    gtfull = nc.dram_tensor("gtfull", [N, GSLOT], fp16, addr_space="Shared")
    arin = nc.dram_tensor("arin", [128, 2], fp32)
    arout = nc.dram_tensor("arout", [128, 2], fp32, addr_space="Shared")

    RG = [list(range(NCORES))]

    with tile.TileContext(nc) as tc:
        import contextlib
        es = contextlib.ExitStack()
        with es:
            pers = es.enter_context(tc.tile_pool(name="pers", bufs=1))
            # ---------- persistent SBUF ----------
            Sres = pers.tile([128, NSLAB, 128], fp8)
            idxs = pers.tile([128, NCH * 8], i16)
            nc.sync.dma_start(idxs[:], idxs_d[:])
            dinv = pers.tile([128, NB], fp32)
            nc.sync.dma_start(dinv[:], dinv_d[:])

            w1 = pers.tile([128, HID], fp16)
            w2 = pers.tile([128, 2 * HID], fp16)
            w3 = pers.tile([128, 2 * H3D], fp16)
            wg = pers.tile([128, HID], fp16)
            wgad = pers.tile([128, 2 * HEADS], fp16)
            b1r = pers.tile([1, HID], fp16)
            b2r = pers.tile([1, HID], fp16)
            b3r = pers.tile([128, H3D], fp32)
            bgr = pers.tile([128, HID], fp16)
            wc1 = pers.tile([128, 2 * 128], fp32)
            wc2 = pers.tile([128, 64], fp32)
            wc3 = pers.tile([64, 8], fp32)
            bc1 = pers.tile([128, 1], fp32)
            bc2 = pers.tile([64, 1], fp32)
            bc3 = pers.tile([8, 1], fp32)
            rowmask = pers.tile([128, 1], fp32)

            def load_persistent():
                nc.sync.dma_start(Sres[:].rearrange("p a b -> p (a b)"), S_d[:])
                for t, d in ((w1, w1_d), (w2, w2_d), (w3, w3_d), (wg, wg_d),
                             (wgad, wgad_d), (b1r, b1_d), (b2r, b2_d),
                             (b3r, b3_d), (bgr, bg_d), (wc1, wc1_d),
                             (wc2, wc2_d), (wc3, wc3_d), (bc1, bc1_d),
                             (bc2, bc2_d), (bc3, bc3_d), (rowmask, rowmask_d)):
                    nc.sync.dma_start(t[:], d[:])

            iota_i = pers.tile([128, 128], i16)
            nc.gpsimd.iota(iota_i[:], pattern=[[1, 128]], base=0, channel_multiplier=0)
            iota_f = pers.tile([128, 128], fp16)
            nc.vector.tensor_copy(iota_f[:], iota_i[:])
            iop_i = pers.tile([128, 1], i16)
            nc.gpsimd.iota(iop_i[:], pattern=[[1, 1]], base=0, channel_multiplier=1)
            iop_f = pers.tile([128, 1], fp16)
            nc.vector.tensor_copy(iop_f[:], iop_i[:])
            ident = pers.tile([128, 128], fp16)
            nc.vector.tensor_tensor(
                ident[:], iop_f[:].broadcast_to([128, 128]), iota_f[:],
                op=ALU.is_equal)
            ones_r = pers.tile([1, 128], fp16)
            nc.vector.memset(ones_r[:], 1.0)
            ones_c = pers.tile([128, 1], fp16)
            nc.vector.memset(ones_c[:], 1.0)

            alad_all = pers.tile([128, NB, 2 * HEADS], fp32)
            als_all = alad_all[:, :, 0:HEADS]
            ald_all = alad_all[:, :, HEADS:2 * HEADS]
            ald16 = pers.tile([128, NB, HEADS], fp16)
            crep = pers.tile([128, HEADS], fp32)

            def rows(b):
                return LASTB if b == NB - 1 else 128

            # ---------- helpers ----------
            def transpose_to_sbuf(pool, psum_pool, src16, nslab, tag):
                """src16 [128, nslab*128] fp16 -> returns [128, nslab, 128] fp16."""
                out = pool.tile([128, nslab, 128], fp16, tag=tag, name=f"tT_{tag}")
                for s in range(nslab):
                    pt = psum_pool.tile([128, 128], fp16, tag="tr", name="pt_tr", bufs=2)
                    nc.tensor.transpose(pt[:], src16[:, s * 128:(s + 1) * 128], ident[:])
                    nc.vector.tensor_copy(out[:, s, :], pt[:])
                return out

            qctr = [0]

            def next_q():
                qctr[0] += 1
                return qctr[0] % 4

            def gather_into(m_tile, table, start_chunk, n_chunks, elem, step):
                # dma_gather clone allowing elem_size < row stride (the 256B
                # multiple constraint only applies to the stride / transpose)
                eng = nc.gpsimd
                in_ap = table
                stride_bytes = step * mybir.dt.size(in_ap.dtype)
                assert stride_bytes % 256 == 0
                _in_ap = eng.lower_ap_dma(in_ap, for_custom_bir_dma=True)
                _idxs_ap = eng.lower_ap(
                    idxs[:, start_chunk * 8:(start_chunk + n_chunks) * 8])
                _out_ap = eng.lower_ap(m_tile[:, 0:n_chunks, :])
                eng.add_instruction(
                    mybir.InstDMAGatherAnt(
                        name=nc.get_next_instruction_name(),
                        ins=[*_in_ap, _idxs_ap,
                             eng.lower_val_access(eng.to_reg(n_chunks * 128))],
                        outs=[_out_ap],
                        transpose=False,
                        num_idxs=n_chunks * 128,
                        elem_size=elem,
                        stride_bytes_256=stride_bytes // 256,
                        gen_mode=0,
                        single_packet=False,
                        queue_num=next_q(),
                    ))

            def maybe_cc(kind, op, replica_groups, ins, outs):
                if no_cc:
                    nc.sync.dma_start(outs[0].tensor[0:ins[0].shape[0]], ins[0])
                else:
                    nc.gpsimd.collective_compute(kind, op, replica_groups=replica_groups,
                                                 ins=ins, outs=outs)

            def agg_matmuls(pagg_of, m_tile, start, n_ch, D, first, done,
                            pair=False):
                """Emit segment matmuls for chunks [start, start+n_ch).
                pair=True fuses same-block chunk pairs with fp8 DoubleRow."""
                kk = 0
                while kk < n_ch:
                    K = start + kk
                    b, si = segs_by_chunk[K][0]
                    if (pair and kk + 1 < n_ch
                            and segs_by_chunk[K + 1][0][0] == b):
                        done[b] += 2
                        nc.tensor.matmul(
                            pagg_of[b][:],
                            Sres[:, si:si + 2, :], m_tile[:, kk:kk + 2, 0:D],
                            start=first[b], stop=(done[b] == total_per_block[b]),
                            perf_mode=mybir.MatmulPerfMode.DoubleRow)
                        first[b] = False
                        kk += 2
                        continue
                    done[b] += 1
                    nc.tensor.matmul(
                        pagg_of[b][:], Sres[:, si, :], m_tile[:, kk, 0:D],
                        start=first[b], stop=(done[b] == total_per_block[b]))
                    first[b] = False
                    kk += 1

            def run_body(rep):
                # ================= phase 0: g1 = dinv * x =================
                NFB = NB - 1          # full 128-row blocks
                with tc.tile_pool(name=f"p0_{rep}", bufs=1) as p0:
                    xt = p0.tile([128, NB, D_IN], fp32, name="xt")
                    nc.sync.dma_start(
                        xt[:, 0:NFB, :],
                        xs[0:NFB * 128, :].rearrange("(b p) d -> p b d", p=128))
                    nc.sync.dma_start(xt[:LASTB, NFB, :], xs[NFB * 128:NPC, :])
                    gt = p0.tile([128, NB, D_IN], fp8, name="gt")
                    nc.vector.tensor_tensor(
                        gt[:], xt[:],
                        dinv[:].unsqueeze(2).broadcast_to([128, NB, D_IN]),
                        op=ALU.mult)
                    nc.scalar.dma_start(
                        g1loc[0:NFB * 128, 0:D_IN].rearrange("(b p) d -> p b d", p=128),
                        gt[:, 0:NFB, :])
                    nc.scalar.dma_start(
                        g1loc[NFB * 128:NPC, 0:D_IN], gt[:LASTB, NFB, :])

                maybe_cc("AllGather", ALU.bypass, RG, [g1loc[:]], [g1full[:]])
                if rep == 0:
                    load_persistent()

                # ================= GCN layer runner =================
                h1_pool = tc.tile_pool(name=f"h1pool_{rep}", bufs=1)
                h1_ctx = h1_pool.__enter__()
                h1_all = h1_ctx.tile([128, NB, HID], fp16)

                def gcn_layer(lname, table_full, Dtab, Dstep, D, evict_fn, mbufs=5, ms=MS):
                    with (tc.tile_pool(name=f"{lname}_sb_{rep}", bufs=2) as lp,
                          tc.tile_pool(name=f"{lname}_ps_{rep}", bufs=4, space="PSUM") as pp,
                          tc.tile_pool(name=f"{lname}_wps_{rep}", bufs=2, space="PSUM") as wp):
                        tab_lo = table_full[0:HALF, :]
                        tab_hi = table_full[HALF:N, :]
                        for (lo_s, lo_n, hi_s, hi_n, blocks) in layout:
                            paggs = {}
                            for b in blocks:
                                paggs[b] = pp.tile([128, D], fp32, tag="agg", name="pagg")
                            first = {b: True for b in blocks}
                            done = {b: 0 for b in blocks}
                            for (start, n_ch, tab) in ((lo_s, lo_n, tab_lo), (hi_s, hi_n, tab_hi)):
                                for s0 in range(0, n_ch, ms):
                                    ns = min(ms, n_ch - s0)
                                    m = lp.tile([128, ns, Dtab], fp8, tag="m", name="m", bufs=mbufs)
                                    gather_into(m, tab, start + s0, ns, Dtab, Dstep)
                                    agg_matmuls(paggs, m, start + s0, ns, D, first, done, pair=PAIR)
                            for b in blocks:
                                evict_fn(b, paggs[b], lp, wp)

                # ---------- layer 1 ----------
                def evict1(b, pagg, lp, wp):
                    r = rows(b)
                    a1s = lp.tile([128, D_IN], fp16, tag="ev1", name="a1s")
                    nc.vector.tensor_scalar(a1s[:], pagg[:], dinv[:, b:b + 1], None, op0=ALU.mult)
                    a1T = transpose_to_sbuf(lp, wp, a1s, 1, "ev1T")
                    ph = wp.tile([128, HID], fp32, tag="wout", name="ph1", bufs=2)
                    nc.tensor.matmul(ph[:], a1T[:, 0, :], w1[:], start=True, stop=False)
                    nc.tensor.matmul(ph[:], ones_r[:], b1r[:], start=False, stop=True)
                    h1t = h1_all[:, b, :]
                    nc.scalar.activation(h1t, ph[:], ACT.Relu)
                    g2t = lp.tile([128, HID], fp8, tag="ev1g", name="g2t")
                    nc.vector.tensor_scalar(g2t[:], h1t, dinv[:, b:b + 1], None, op0=ALU.mult)
                    nc.sync.dma_start(g2loc[b * 128:b * 128 + r, :], g2t[:r, :])

                gcn_layer("L1", g1full, D_IN, 256, D_IN, evict1, ms=MS13)
                maybe_cc("AllGather", ALU.bypass, RG, [g2loc[:]], [g2full[:]])

                # ---------- layer 2 (+ residual + L3 transform) ----------
                def evict2(b, pagg, lp, wp):
                    r = rows(b)
                    a2s = lp.tile([128, HID], fp16, tag="ev2", name="a2s")
                    nc.vector.tensor_scalar(a2s[:], pagg[:], dinv[:, b:b + 1], None, op0=ALU.mult)
                    a2T = transpose_to_sbuf(lp, wp, a2s, 2, "ev2T")
                    ph = wp.tile([128, HID], fp32, tag="wout", name="ph2", bufs=2)
                    nc.tensor.matmul(ph[:], a2T[:, 0, :], w2[:, 0:HID], start=True, stop=False)
                    nc.tensor.matmul(ph[:], a2T[:, 1, :], w2[:, HID:2 * HID], start=False, stop=False)
                    nc.tensor.matmul(ph[:], ones_r[:], b2r[:], start=False, stop=True)
                    r2 = lp.tile([128, HID], fp16, tag="ev2r", name="r2")
                    nc.scalar.activation(r2[:], ph[:], ACT.Relu)
                    h2t16 = lp.tile([128, HID], fp16, tag="ev2h6", name="h2t16")
                    nc.vector.tensor_tensor(h2t16[:], r2[:], h1_all[:, b, :], op=ALU.add)
                    h2T = transpose_to_sbuf(lp, wp, h2t16, 2, "ev2hT")
                    pt3 = wp.tile([128, H3D], fp32, tag="wout", name="pt3", bufs=2)
                    nc.tensor.matmul(pt3[:], h2T[:, 0, :], w3[:, 0:H3D], start=True, stop=False)
                    nc.tensor.matmul(pt3[:], h2T[:, 1, :], w3[:, H3D:2 * H3D], start=False, stop=True)
                    g3t = lp.tile([128, H3D], fp8, tag="ev2g", name="g3t")
                    nc.vector.tensor_scalar(g3t[:], pt3[:], dinv[:, b:b + 1], None, op0=ALU.mult)
                    nc.sync.dma_start(g3loc[b * 128:b * 128 + r, 0:H3D], g3t[:r, :])

                gcn_layer("L2", g2full, HID, HID, HID, evict2, mbufs=4)
                h1_pool.__exit__(None, None, None)
                maybe_cc("AllGather", ALU.bypass, RG, [g3loc[:]], [g3full[:]])

                # ---------- layer 3 aggregation + GAT prep ----------
                def evict3(b, pagg, lp, wp):
                    r = rows(b)
                    a3s = lp.tile([128, H3D], fp32, tag="ev3", name="a3s")
                    nc.vector.tensor_scalar(a3s[:], pagg[:], dinv[:, b:b + 1], None, op0=ALU.mult)
                    a3b = lp.tile([128, H3D], fp32, tag="ev3b", name="a3b")
                    nc.vector.tensor_tensor(a3b[:], a3s[:], b3r[:], op=ALU.add)
                    h3t16 = lp.tile([128, H3D], fp16, tag="ev3h", name="h3t16")
                    nc.scalar.activation(h3t16[:], a3b[:], ACT.Relu)
                    h3T = transpose_to_sbuf(lp, wp, h3t16, 1, "ev3T")
                    # hg plus al_s / al_d (Wg @ a_src/a_dst) in one psum tile
                    phg = wp.tile([128, HID + 2 * HEADS], fp32, tag="wout", name="phg", bufs=2)
                    nc.tensor.matmul(phg[:, 0:HID], h3T[:, 0, :], wg[:], start=True, stop=True)
                    nc.tensor.matmul(phg[:, HID:], h3T[:, 0, :], wgad[:], start=True, stop=True)
                    nc.vector.tensor_copy(alad_all[:, b, :], phg[:, HID:])
                    # table tile: [hg fp16 (f,h) | als fp16] (row pad never written)
                    tabt = lp.tile([128, DM], fp16, tag="ev3tab", name="tabt")
                    nc.scalar.copy(tabt[:, 0:HID], phg[:, 0:HID])
                    nc.scalar.copy(tabt[:, HID:HID + HEADS], phg[:, HID:HID + HEADS])
                    nc.sync.dma_start(gtloc[b * 128:b * 128 + r, 0:DM], tabt[:r, :])

                gcn_layer("L3", g3full, H3D, 256, H3D, evict3, ms=MS13)
                nc.vector.tensor_copy(ald16[:], ald_all)

                # shift constants c[h] = leaky(max al_s + max al_d)
                cps = contextlib.ExitStack()
                cp = cps.enter_context(tc.tile_pool(name=f"cp_{rep}", bufs=1))
                cpp = cps.enter_context(tc.tile_pool(name=f"cpp_{rep}", bufs=1, space="PSUM"))
                m1 = cp.tile([128, HEADS], fp32)
                nc.vector.tensor_reduce(
                    m1[:], als_all.rearrange("p b h -> p h b"),
                    axis=mybir.AxisListType.X, op=ALU.max)
                m2 = cp.tile([128, HEADS], fp32)
                nc.vector.tensor_reduce(
                    m2[:], ald_all.rearrange("p b h -> p h b"),
                    axis=mybir.AxisListType.X, op=ALU.max)
                m1_16 = cp.tile([128, HEADS], fp16)
                nc.vector.tensor_copy(m1_16[:], m1[:])
                m2_16 = cp.tile([128, HEADS], fp16)
                nc.vector.tensor_copy(m2_16[:], m2[:])
                pmt1 = cpp.tile([HEADS, 128], fp16, tag="pmt1", name="pmt1")
                nc.tensor.transpose(pmt1[:], m1_16[:], ident[:])
                pmt2 = cpp.tile([HEADS, 128], fp16, tag="pmt2", name="pmt2")
                nc.tensor.transpose(pmt2[:], m2_16[:], ident[:])
                mt = cp.tile([HEADS, 2 * 128], fp32)
                nc.scalar.copy(mt[:, 0:128], pmt1[:])
                nc.scalar.copy(mt[:, 128:256], pmt2[:])
                ms = cp.tile([HEADS, 2], fp32)
                nc.vector.tensor_reduce(
                    ms[:], mt[:].rearrange("p (a j) -> p a j", a=2),
                    axis=mybir.AxisListType.X, op=ALU.max)
                ub = cp.tile([HEADS, 1], fp32)
                nc.vector.tensor_tensor(ub[:], ms[:, 0:1], ms[:, 1:2], op=ALU.add)
                ub2 = cp.tile([HEADS, 1], fp32)
                nc.vector.tensor_scalar(ub2[:], ub[:], 0.2, None, op0=ALU.mult)
                cc = cp.tile([HEADS, 1], fp32)
                nc.vector.tensor_tensor(cc[:], ub[:], ub2[:], op=ALU.max)
                cc16 = cp.tile([HEADS, 1], fp16)
                nc.vector.tensor_copy(cc16[:], cc[:])
                pcr = cpp.tile([1, HEADS], fp16)
                nc.tensor.transpose(pcr[:], cc16[:HEADS, :], ident[0:HEADS, 0:HEADS])
                pcr_sb = cp.tile([1, HEADS], fp16)
                nc.scalar.copy(pcr_sb[:], pcr[:])
                pcrep = cpp.tile([128, HEADS], fp32)
                nc.tensor.matmul(pcrep[:], ones_r[:], pcr_sb[:], start=True, stop=True)
                nc.scalar.copy(crep[:], pcrep[:])
                cps.close()

                maybe_cc("AllGather", ALU.bypass, RG, [gtloc[:]], [gtfull[:]])

                # ================= GAT layer =================
                plp_cm = tc.tile_pool(name=f"pool_ps_{rep}", bufs=1, space="PSUM")
                plp = plp_cm.__enter__()
                ppool = plp.tile([128, 2], fp32, tag="pp", name="ppool")
                ppool0 = ppool[:, 0:1]
                ppool1 = ppool[:, 1:2]
                with (tc.tile_pool(name=f"gat_sb_{rep}", bufs=2) as gp,
                      tc.tile_pool(name=f"gat_ps_{rep}", bufs=4, space="PSUM") as gpp,
                      tc.tile_pool(name=f"gat_ups_{rep}", bufs=3, space="PSUM") as upp):
                    tab_lo = gtfull[0:HALF, :]
                    tab_hi = gtfull[HALF:N, :]
                    first = {b: True for b in range(NB)}
                    done = {b: 0 for b in range(NB)}
                    paggs = {}

                    def gat_span(start, n_ch, tab):
                        if n_ch == 0:
                            return
                        m = gp.tile([128, n_ch, DM], fp16, tag="gm", name="gm", bufs=6)
                        gather_into(m, tab, start, n_ch, DM, GSLOT)
                        # stream S_T slabs for this span
                        si0 = segs_by_chunk[start][0][1]
                        si1 = segs_by_chunk[start + n_ch - 1][-1][1] + 1
                        nsl = si1 - si0
                        st = gp.tile([128, nsl, 128], fp8, tag="gst", name="gst", bufs=4)
                        nc.sync.dma_start(
                            st[:].rearrange("p a b -> p (a b)"),
                            ST_d[:, si0 * 128:si1 * 128])
                        # u[e, h] = als[e, h] + ald[dst_e, h] via PE
                        ups = upp.tile([128, n_ch, HEADS], fp32, tag="ups", name="ups")
                        nc.tensor.matmul(
                            ups[:], ident[:], m[:, :, HID:HID + HEADS],
                            start=True, stop=False, skip_group_check=True)
                        for kk in range(n_ch):
                            segs = segs_by_chunk[start + kk]
                            for j, (b, si) in enumerate(segs):
                                nc.tensor.matmul(
                                    ups[:, kk, :], st[:, si - si0, :], ald16[:, b, :],
                                    start=False, stop=(j == len(segs) - 1),
                                    skip_group_check=True)
                        # leaky_relu on DVE (keeps ACT on the exp table), shift
                        lr2 = gp.tile([128, n_ch, HEADS], fp32, tag="glr2", name="glr2")
                        nc.vector.tensor_scalar(
                            lr2[:], ups[:], NEG, None, op0=ALU.mult)
                        lr = gp.tile([128, n_ch, HEADS], fp32, tag="glr", name="glr")
                        nc.vector.tensor_tensor(lr[:], ups[:], lr2[:], op=ALU.max)
                        lsh = gp.tile([128, n_ch, HEADS], fp32, tag="glsh", name="glsh")
                        nc.vector.tensor_tensor(
                            lsh[:], lr[:],
                            crep[:].unsqueeze(1).broadcast_to([128, n_ch, HEADS]),
                            op=ALU.subtract)
                        # exp weights straight into the den message cols
                        nc.scalar.activation(
                            m[:, :, HID + HEADS:HID + 2 * HEADS],
                            lsh[:].rearrange("p k h -> p (k h)"), ACT.Exp)
                        nc.vector.tensor_tensor(
                            m[:, :, 0:HID].rearrange("p k (f h) -> p k f h", h=HEADS),
                            m[:, :, 0:HID].rearrange("p k (f h) -> p k f h", h=HEADS),
                            m[:, :, HID + HEADS:HID + 2 * HEADS]
                                .unsqueeze(2).broadcast_to([128, n_ch, FH, HEADS]),
                            op=ALU.mult)
                        agg_matmuls(paggs, m, start, n_ch, DM, first, done)

                    def gat_evict(b):
                        r = rows(b)
                        pg = paggs.pop(b)
                        den = gp.tile([128, HEADS], fp32, tag="gden", name="gden")
                        nc.scalar.copy(den[:], pg[:, HID + HEADS:HID + 2 * HEADS])
                        nc.vector.tensor_scalar(den[:], den[:], 1e-30, None, op0=ALU.max)
                        rden = gp.tile([128, HEADS], fp32, tag="grden", name="grden")
                        nc.vector.reciprocal(rden[:], den[:])
                        t1 = gp.tile([128, HID], fp16, tag="gt1", name="gt1")
                        nc.vector.tensor_tensor(
                            t1[:].rearrange("p (f h) -> p f h", h=HEADS),
                            pg[:, 0:HID].rearrange("p (f h) -> p f h", h=HEADS),
                            rden[:].unsqueeze(1).broadcast_to([128, FH, HEADS]),
                            op=ALU.mult)
                        t2 = gp.tile([128, HID], fp16, tag="gt2", name="gt2")
                        nc.vector.tensor_tensor(t2[:], t1[:], bgr[:], op=ALU.add)
                        hatt = gp.tile([128, HID], fp16, tag="ghat", name="ghat")
                        nc.scalar.activation(hatt[:], t2[:], ACT.Relu)
                        if r < 128:
                            nc.vector.tensor_scalar(hatt[:], hatt[:], rowmask[:], None, op0=ALU.mult)
                        nc.tensor.matmul(ppool0, hatt[:, 0:128], ones_c[:],
                                         start=(b == 0), stop=(b == NB - 1))
                        nc.tensor.matmul(ppool1, hatt[:, 128:256], ones_c[:],
                                         start=(b == 0), stop=(b == NB - 1))

                    for (lo_s, lo_n, hi_s, hi_n, blocks) in layout:
                        for b in blocks:
                            paggs[b] = gpp.tile([128, DM], fp32, tag="gagg", name="gagg")
                        for (start, n_ch, tab) in ((lo_s, lo_n, tab_lo), (hi_s, hi_n, tab_hi)):
                            for s0 in range(0, n_ch, MS_GAT):
                                gat_span(start + s0, min(MS_GAT, n_ch - s0), tab)
                        for b in blocks:
                            gat_evict(b)

                # ---------- pooling + AllReduce + MLP ----------
                with (tc.tile_pool(name=f"mlp_sb_{rep}", bufs=1) as mp,
                      tc.tile_pool(name=f"mlp_ps_{rep}", bufs=1, space="PSUM") as mpp):
                    pool_sb = mp.tile([128, 2], fp32, name="pool_sb")
                    nc.scalar.copy(pool_sb[:], ppool[:])
                    nc.sync.dma_start(arin[:], pool_sb[:])
                    maybe_cc("AllReduce", ALU.add, RG, [arin[:]], [arout[:]])
                    pooled = mp.tile([128, 2], fp32, name="pooled")
                    nc.sync.dma_start(pooled[:], arout[:])
                    nc.vector.tensor_scalar(pooled[:], pooled[:], 1.0 / N, None, op0=ALU.mult)
                    pz1 = mpp.tile([128, 1], fp32, tag="pz", name="pz1")
                    nc.tensor.matmul(pz1[:], wc1[:, 0:128], pooled[:, 0:1], start=True, stop=False)
                    nc.tensor.matmul(pz1[:], wc1[:, 128:256], pooled[:, 1:2], start=False, stop=True)
                    z1 = mp.tile([128, 1], fp32, name="z1")
                    nc.scalar.activation(z1[:], pz1[:], ACT.Relu, bias=bc1[:])
                    pz2 = mpp.tile([64, 1], fp32, tag="pz", name="pz2")
                    nc.tensor.matmul(pz2[:], wc2[:], z1[:], start=True, stop=True)
                    z2 = mp.tile([64, 1], fp32, name="z2")
                    nc.scalar.activation(z2[:], pz2[:], ACT.Relu, bias=bc2[:])
                    pz3 = mpp.tile([8, 1], fp32, tag="pz", name="pz3")
                    nc.tensor.matmul(pz3[:], wc3[:], z2[:64, :], start=True, stop=True)
                    zo = mp.tile([8, 1], fp32, name="zo")
                    nc.scalar.activation(zo[:], pz3[:], ACT.Identity, bias=bc3[:])
                    nc.sync.dma_start(out_d[:], zo[:])
                plp_cm.__exit__(None, None, None)

            for _rep in range(repeat):
                run_body(_rep)

    nc.compile()
    return nc


# --------------------------------------------------------------------------
# entry point
# --------------------------------------------------------------------------

def _fh_interleave(w):
    """Reorder columns from (h, f) to (f, h): col f*H+h <- col h*FH+f."""
    M = w.reshape(w.shape[0], HEADS, FH)
    return np.ascontiguousarray(M.transpose(0, 2, 1).reshape(w.shape[0], HEADS * FH))


def kernel(**inputs):
    x = np.asarray(inputs["x"], dtype=np.float32)
    ei = np.asarray(inputs["edge_index"], dtype=np.int64)
    sched = _preprocess(x, ei)
    nc = _build(sched)

    W = {k: np.asarray(v, dtype=np.float32) for k, v in inputs.items()
         if k not in ("x", "edge_index")}

    def pack_k(w, nslab):   # [K, M] -> [128, nslab*M] (row-slab packed)
        K, M = w.shape
        out = np.zeros((128, nslab * M), np.float32)
        for s in range(nslab):
            r0 = s * 128
            r1 = min(K, r0 + 128)
            out[0:r1 - r0, s * M:(s + 1) * M] = w[r0:r1]
        return out

    # (f, h)-interleaved GAT weights
    wg_i = _fh_interleave(W["Wg"])                              # [128, 256]
    bg_i = _fh_interleave(W["bg"].reshape(1, HEADS * FH))
    wc1_i = _fh_interleave(W["Wc1"].T).T                        # rows reordered

    common = {
        "w1_d": pack_k(W["W1"], 1).astype(F16),
        "w2_d": pack_k(W["W2"], 2).astype(F16),
        "w3_d": pack_k(W["W3"], 2).astype(F16),
        "wg_d": pack_k(wg_i, 1).astype(F16),
        "b1_d": W["b1"].reshape(1, -1).astype(F16),
        "b2_d": W["b2"].reshape(1, -1).astype(F16),
        "b3_d": np.tile(W["b3"].reshape(1, -1), (128, 1)).astype(np.float32),
        "bg_d": np.tile(bg_i, (128, 1)).astype(F16),
        "wgad_d": np.concatenate([
            (W["Wg"].reshape(D_IN, HEADS, FH) * W["a_src"][None]).sum(-1),
            (W["Wg"].reshape(D_IN, HEADS, FH) * W["a_dst"][None]).sum(-1),
        ], axis=1).astype(F16),
        "wc1_d": pack_k(wc1_i, 2).astype(np.float32),
        "wc2_d": pack_k(W["Wc2"], 1)[:, :64].astype(np.float32),
        "wc3_d": pack_k(W["Wc3"], 1)[:64, :8].astype(np.float32),
        "bc1_d": W["bc1"].reshape(-1, 1).astype(np.float32),
        "bc2_d": W["bc2"].reshape(-1, 1).astype(np.float32),
        "bc3_d": W["bc3"].reshape(-1, 1).astype(np.float32),
        "rowmask_d": (np.arange(128) < LASTB).astype(np.float32).reshape(128, 1),
    }

    in_maps = []
    for c in range(NCORES):
        in_maps.append(dict(
            common,
            xs=np.ascontiguousarray(x[c * NPC:(c + 1) * NPC]),
            idxs_d=sched["idxs"][c],
            S_d=sched["S"][c],
            ST_d=sched["ST"][c],
            dinv_d=sched["dinv"][c],
        ))

    res = run_bass_kernel_spmd(nc, in_maps, core_ids=list(range(NCORES)))
    global LAST_RESULT
    LAST_RESULT = res
    return res.results[0]["out_d"].reshape(1, OUT).astype(np.float32)


LAST_RESULT = None
